# revision 27
# baseline (speedup 1.0000x reference)
"""Trainium2 Bass kernel for nn_LinearAttention (gated linear attention).

Math (per reference):
    qkv = x @ Wqkv.T ; q,k,v = split(qkv); q,k = elu(.)+1
    per (b,h): running_kv[t]  = d*running_kv[t-1]  + k[t]*v[t]   (elementwise, D=64)
               running_ksum[t]= d*running_ksum[t-1]+ k[t]
    den = clip(sum_d(q*running_ksum), 1e-6); out = q*running_kv/den
    g = sigmoid(out @ Wgate.T + bgate); out = g*out + (1-g)*v
    y = out @ Wout.T

Implementation strategy (8 NeuronCores, SPMD, no collectives):
  - Token-parallel: core c handles batch b=c//2, T-half h=c%2 (2048 tokens)
    plus a 128-token halo before the chunk to warm the decay scan
    (decay=0.95 => truncation error ~0.95^128 ~ 1.4e-3).  Half 0 gets a
    zero halo + k-mask so its scan state is exactly 0 at t=0.
  - Everything on-chip lives as [feature(partition), token(free)]; the host
    pre-transposes x and the weight matrices so both matmul operands are in
    natural layout and no on-chip transpose is ever needed.  The final
    output is produced transposed ([hidden, T]) and un-transposed on host.
  - The decay scans run natively on DVE via tensor_tensor_scan (state =
    d*state + u along the free/time axis), chained across groups through
    small fp32 state tiles.  The kv-scan init reads 0*mix+state so the
    Tile scheduler orders the scan train after the mix chain on the
    in-order DVE queue (the y matmuls would otherwise stall on mix).
    Pool (gpsimd) runs only plain tensor_tensor ops (qckv/prods/dl/kvs;
    it has no PSUM port and rejects scans/tensor_scalar at codegen).
  - phi(x)=elu(x)+1 = exp(min(x,0)) + relu(x), computed as
      rm = ACT.Relu(-x) (from PSUM), qe = ACT.Exp(-rm),
      q1 = DVE.stt((x max 0) add qe) (from PSUM)
    so the PSUM tile is never copied to SBUF first.
  - den: sum over D=64 partitions via a 0/1 selector matmul (PSUM [16,WG]);
    clip + reciprocal_approx_fast; broadcast back to 128 partitions via a
    second selector matmul in fp32r.
  - gate: sigmoid(L) = 0.5*tanh(0.5*L) + 0.5 so the ACT engine only ever
    needs the exp/relu/tanh/copy activation table (no table swaps); the
    0.5 factors fold into the mix scalar_tensor_tensor ops:
      mix = g*oa + (1-g)*v = oa + 0.5*((th-1)*(oa-v))  where th=tanh(L/2).
"""

import sys

for _p in ('/opt/trn_rl_repo', '/root/.axon_site'):
    if _p not in sys.path:
        sys.path.insert(0, _p)

from contextlib import ExitStack

import ml_dtypes
import numpy as np

import concourse.tile as tile
from concourse import bacc, mybir
from concourse.bass_utils import run_bass_kernel_spmd

F32 = mybir.dt.float32
F32R = mybir.dt.float32r
BF16 = mybir.dt.bfloat16
AL = mybir.AluOpType
AF = mybir.ActivationFunctionType

B, T, HID = 4, 4096, 1024
H, D = 16, 64
OD = 3 * HID              # 3072 qkv output rows
NK = HID // 128           # 8 hidden (contraction) tiles
HALF_T = T // 2           # 2048 tokens per core
HALO = 128
TLOC = HALO + HALF_T      # 2176
NH = HID // 128           # 8 tiles per q/k/v section
# groups: (token offset in TLOC, width); group 0 is the halo
GROUPS = [(0, HALO)] + [(HALO + 512 * i, 512) for i in range(4)]
NG = len(GROUPS)

_cache = {}


def _build_nc():
    nc = bacc.Bacc("TRN2", target_bir_lowering=False, debug=False)

    xT = nc.dram_tensor("xT", [HID, TLOC], BF16, kind="ExternalInput")
    wqkvT = nc.dram_tensor("wqkvT", [HID, OD], BF16, kind="ExternalInput")
    wgateT = nc.dram_tensor("wgateT", [HID, HID], BF16, kind="ExternalInput")
    woutT = nc.dram_tensor("woutT", [HID, HID], BF16, kind="ExternalInput")
    dec_c = nc.dram_tensor("dec_c", [128, NH], F32, kind="ExternalInput")
    mask_c = nc.dram_tensor("mask_c", [128, 1], F32, kind="ExternalInput")
    densel = nc.dram_tensor("densel", [128, NH * H], BF16, kind="ExternalInput")
    bcsel = nc.dram_tensor("bcsel", [H, NH * 128], F32R, kind="ExternalInput")
    bgate_c = nc.dram_tensor("bgate_c", [128, NH], F32, kind="ExternalInput")
    yT = nc.dram_tensor("yT", [HID, HALF_T], BF16, kind="ExternalOutput")

    with tile.TileContext(nc) as tc, ExitStack() as ctx:
        consts = ctx.enter_context(tc.tile_pool(name="consts", bufs=1))
        wq_pool = ctx.enter_context(tc.tile_pool(name="wq", bufs=1))
        wg_pool = ctx.enter_context(tc.tile_pool(name="wgp", bufs=1))
        wo_pool = ctx.enter_context(tc.tile_pool(name="wop", bufs=1))
        xt_pool = ctx.enter_context(tc.tile_pool(name="xt", bufs=16))
        xth_pool = ctx.enter_context(tc.tile_pool(name="xth", bufs=8))
        qkv_pool = ctx.enter_context(tc.tile_pool(name="qkv", bufs=9))
        tmp_pool = ctx.enter_context(tc.tile_pool(name="tmp", bufs=2))
        cum_pool = ctx.enter_context(tc.tile_pool(name="cum", bufs=1))
        st_pool = ctx.enter_context(tc.tile_pool(name="st", bufs=2))
        oa_pool = ctx.enter_context(tc.tile_pool(name="oa", bufs=8))
        gt_pool = ctx.enter_context(tc.tile_pool(name="gt", bufs=2))
        mix_pool = ctx.enter_context(tc.tile_pool(name="mix", bufs=8))
        y_pool = ctx.enter_context(tc.tile_pool(name="ysb", bufs=2))
        ps_pool = ctx.enter_context(tc.tile_pool(name="ps", bufs=7, space="PSUM"))
        psd_pool = ctx.enter_context(tc.tile_pool(name="psd", bufs=1, space="PSUM"))

        # ---- constants + weights (split across sync/scalar DMA queues) ----
        dec_s = consts.tile([128, NH], F32, tag="dec")
        mask_s = consts.tile([128, 1], F32, tag="mask")
        densel_s = consts.tile([128, NH * H], BF16, tag="densel")
        bcsel_s = consts.tile([H, NH * 128], F32R, tag="bcsel")
        bgate_s = consts.tile([128, NH], F32, tag="bg")

        wq_sec = {}
        for sec in range(3):
            wq_sec[sec] = [
                wq_pool.tile([128, HID], BF16, tag=f"wq{sec}_{k}",
                             name=f"wq_{sec}_{k}") for k in range(NK)]
        wg_s = [wg_pool.tile([128, HID], BF16, tag=f"wg{k}", name=f"wg_{k}")
                for k in range(NK)]
        wo_s = [wo_pool.tile([128, HID], BF16, tag=f"wo{k}", name=f"wo_{k}")
                for k in range(NK)]

        def load_w(eng, sec, k):
            eng.dma_start(
                wq_sec[sec][k][:],
                wqkvT.ap()[128 * k:128 * (k + 1), HID * sec:HID * (sec + 1)])

        # gpsimd queue: odd k-weights first (first matmul chain needs all
        # of w1), then consts (halo scans need dec/mask by ~13us), then
        # q/gate/out weights.  Keeping these off the scalar queue keeps
        # ACT free for the phi chain.
        for k in range(1, NK, 2):
            load_w(nc.gpsimd, 1, k)
        nc.gpsimd.dma_start(dec_s[:], dec_c.ap()[:, :])
        nc.gpsimd.dma_start(mask_s[:], mask_c.ap()[:, :])

        def emit_xt(g):
            off, wg = GROUPS[g]
            tok = slice(off, off + wg)
            xts = []
            for k in range(NK):
                pool, tg = (xth_pool, "xth") if g == 0 else (xt_pool, "xt")
                xt_t = pool.tile([128, wg], BF16, tag=tg, name=f"xt_{g}_{k}")
                nc.sync.dma_start(xt_t[:], xT.ap()[128 * k:128 * (k + 1), tok])
                xts.append(xt_t)
            return xts

        # Critical startup DMAs spread over four queues so the first
        # k-matmul chain (all of w1 + xt0) completes as early as possible:
        #   sync:   w1 evens, xt0 0-3, xt1 0-3
        #   scalar: xt0 4-7, xt1 4-7
        #   scalar: also w2 evens;  gpsimd: w1 odds, consts, w2 odds,
        #   then w0, wgate, wout
        off0, wg0 = GROUPS[0]
        off1, wg1 = GROUPS[1]

        def emit_x(pool, tg, g, off, wgx, k, eng):
            t = pool.tile([128, wgx], BF16, tag=tg, name=f"xt_{g}_{k}")
            eng.dma_start(t[:], xT.ap()[128 * k:128 * (k + 1), off:off + wgx])
            return t

        xts0 = [None] * NK
        xts_g1 = [None] * NK
        load_w(nc.sync, 1, 0)
        load_w(nc.sync, 1, 2)
        xts0[0] = emit_x(xth_pool, "xth", 0, off0, wg0, 0, nc.sync)
        xts0[1] = emit_x(xth_pool, "xth", 0, off0, wg0, 1, nc.sync)
        load_w(nc.sync, 1, 4)
        load_w(nc.sync, 1, 6)
        xts0[2] = emit_x(xth_pool, "xth", 0, off0, wg0, 2, nc.sync)
        xts0[3] = emit_x(xth_pool, "xth", 0, off0, wg0, 3, nc.sync)
        for k in range(4, NK):
            xts0[k] = emit_x(xth_pool, "xth", 0, off0, wg0, k, nc.scalar)
        for k in range(0, NK, 2):
            load_w(nc.scalar, 2, k)
        for k in range(1, NK, 2):
            load_w(nc.gpsimd, 2, k)
        for k in range(NK):
            eng = nc.sync if k < 4 else nc.scalar
            xts_g1[k] = emit_x(xt_pool, "xt", 1, off1, wg1, k, eng)
        nc.gpsimd.dma_start(bgate_s[:], bgate_c.ap()[:, :])
        nc.gpsimd.dma_start(densel_s[:], densel.ap()[:, :])
        nc.gpsimd.dma_start(bcsel_s[:], bcsel.ap()[:, :])
        for k in range(NK):
            load_w(nc.gpsimd, 0, k)
        for k in range(NK):
            nc.gpsimd.dma_start(
                wg_s[k][:], wgateT.ap()[128 * k:128 * (k + 1), :])
            nc.gpsimd.dma_start(
                wo_s[k][:], woutT.ap()[128 * k:128 * (k + 1), :])

        # ---- per-group emitters -----------------------------------------
        def emit_qkv(g, xts, sec, q1, k1, vv):
            """PE: qkv matmuls for one section; ACT/DVE: phi or v drains."""
            _, wg = GROUPS[g]
            is_halo = g == 0
            for oti in range(NH):
                ps = ps_pool.tile([128, wg], F32, tag="mm",
                                  name=f"qkvp_{g}_{sec}_{oti}")
                for k in range(NK):
                    nc.tensor.matmul(
                        ps[:], wq_sec[sec][k][:, 128 * oti:128 * (oti + 1)],
                        xts[k][:], start=(k == 0), stop=(k == NK - 1))
                if sec < 2:  # q or k: phi = exp(min(x,0)) + relu(x)
                    rm = tmp_pool.tile([128, wg], BF16, tag="rm", bufs=1,
                                       name=f"rm_{g}_{sec}_{oti}")
                    nc.scalar.activation(rm[:], ps[:], AF.Relu, scale=-1.0)
                    qe = tmp_pool.tile([128, wg], BF16, tag="qe", bufs=2,
                                       name=f"qe_{g}_{sec}_{oti}")
                    nc.scalar.activation(qe[:], rm[:], AF.Exp, scale=-1.0)
                    if sec == 0:
                        q1[oti] = qkv_pool.tile([128, wg], BF16, tag="q1",
                                                name=f"q1_{g}_{oti}")
                        nc.vector.scalar_tensor_tensor(
                            q1[oti][:], ps[:], 0.0, qe[:], AL.max, AL.add)
                    elif is_halo:
                        kr = tmp_pool.tile([128, wg], BF16, tag="kraw", bufs=1,
                                           name=f"kr_{g}_{oti}")
                        nc.vector.scalar_tensor_tensor(
                            kr[:], ps[:], 0.0, qe[:], AL.max, AL.add)
                        k1[oti] = qkv_pool.tile([128, wg], BF16, tag="k1",
                                                name=f"k1_{g}_{oti}")
                        nc.vector.tensor_scalar_mul(
                            k1[oti][:], kr[:], mask_s[:, 0:1])
                    else:
                        k1[oti] = qkv_pool.tile([128, wg], BF16, tag="k1",
                                                name=f"k1_{g}_{oti}")
                        nc.vector.scalar_tensor_tensor(
                            k1[oti][:], ps[:], 0.0, qe[:], AL.max, AL.add)
                else:  # v
                    vv[oti] = qkv_pool.tile([128, wg], BF16, tag="v", bufs=8,
                                            name=f"v_{g}_{oti}")
                    nc.scalar.copy(vv[oti][:], ps[:])

        state = {"ks": [None] * NH, "kv": [None] * NH}

        def emit_ksum_scans(g, k1):
            _, wg = GROUPS[g]
            cum_ks = [None] * NH
            for j in range(NH):
                dec_b = dec_s[:, j:j + 1].broadcast_to([128, wg])
                cum_ks[j] = cum_pool.tile([128, wg], BF16, tag=f"cks{j}",
                                          name=f"cks_{g}_{j}")
                init = 0.0 if g == 0 else state["ks"][j][:, 0:1]
                nc.vector.tensor_tensor_scan(
                    cum_ks[j][:], dec_b, k1[j][:], init, AL.mult, AL.add)
            if g < NG - 1:
                nks = [None] * NH
                for j in range(NH):
                    nks[j] = st_pool.tile([128, 1], F32, tag=f"sks{j}",
                                          name=f"sks_{g}_{j}")
                    nc.vector.tensor_copy(nks[j][:], cum_ks[j][:, wg - 1:wg])
                state["ks"] = nks
            return cum_ks

        def emit_prods(g, q1, cum_ks):
            _, wg = GROUPS[g]
            prods = [None] * NH
            for j in range(NH):
                prods[j] = tmp_pool.tile([128, wg], BF16, tag="prod",
                                         bufs=8, name=f"prod_{g}_{j}")
                nc.gpsimd.tensor_mul(prods[j][:], q1[j][:], cum_ks[j][:])
            return prods

        def emit_den_mm(g, prods):
            _, wg = GROUPS[g]
            dps = psd_pool.tile([H, wg], F32, tag="den", name=f"dps_{g}")
            for j in range(NH):
                nc.tensor.matmul(
                    dps[:], densel_s[:, H * j:H * (j + 1)], prods[j][:],
                    start=(j == 0), stop=(j == NH - 1))
            return dps

        def emit_den_recip(g, dps):
            # ~51-ULP reciprocal straight from the PSUM accumulator into an
            # fp32r tile (identical bit layout to fp32) in a single DVE op.
            # den = sum of strictly-positive products (phi > 0), so the
            # reference's 1e-6 clip can never bind and is skipped.
            from concourse.dve_ops import (
                RECIP_APPROX_FAST_CONSTS,
                RECIPROCAL_APPROX_FAST,
            )
            _, wg = GROUPS[g]
            den_ir = tmp_pool.tile([H, wg], F32R, tag="denir",
                                   name=f"denir_{g}")
            c = RECIP_APPROX_FAST_CONSTS
            nc.vector._custom_dve(
                RECIPROCAL_APPROX_FAST, out=den_ir[:], in0=dps[:],
                s0=c["s0"], s1=c["s1"], imm2=c["imm2"])
            return den_ir

        def emit_kvs(g, k1, vv):
            """kv product on Pool, written in place over k1 (its last
            reader) so Pool never waits on a fresh buffer."""
            for j in range(NH):
                nc.gpsimd.tensor_mul(k1[j][:], k1[j][:], vv[j][:])
            return k1

        def emit_kv_scans(g, kvs, mix_dep=None):
            _, wg = GROUPS[g]
            cum_kv = [None] * NH
            for j in range(NH):
                dec_b = dec_s[:, j:j + 1].broadcast_to([128, wg])
                cum_kv[j] = cum_pool.tile([128, wg], BF16, tag=f"ckv{j}",
                                          name=f"ckv_{g}_{j}")
                if g == 0:
                    init = 0.0
                elif mix_dep is not None:
                    # init = 0*mix + state: value-neutral read of mix[j]
                    # that forces the scheduler to order this scan train
                    # after the mix chain (whose consumer is the in-order
                    # PE y matmul) on the shared DVE queue.
                    st2 = st_pool.tile([128, 1], F32, tag=f"sk2{j}",
                                       name=f"st2_{g}_{j}")
                    nc.vector.scalar_tensor_tensor(
                        st2[:], mix_dep[NH - 1][:, 0:1], 0.0,
                        state["kv"][j][:], AL.mult, AL.add)
                    init = st2[:, 0:1]
                else:
                    init = state["kv"][j][:, 0:1]
                nc.vector.tensor_tensor_scan(
                    cum_kv[j][:], dec_b, kvs[j][:], init, AL.mult, AL.add)
            if g < NG - 1:
                nkv = [None] * NH
                for j in range(NH):
                    nkv[j] = st_pool.tile([128, 1], F32, tag=f"skv{j}",
                                          name=f"skv_{g}_{j}")
                    nc.vector.tensor_copy(nkv[j][:], cum_kv[j][:, wg - 1:wg])
                state["kv"] = nkv
            return cum_kv

        def emit_oa(g, q1, cum_kv, den_ir):
            """bc broadcast matmuls, qckv mults (Pool), attention out."""
            _, wg = GROUPS[g]
            qckv = [None] * NH
            for j in range(NH):
                qckv[j] = tmp_pool.tile([128, wg], BF16, tag="qckv", bufs=3,
                                        name=f"qckv_{g}_{j}")
                nc.gpsimd.tensor_mul(qckv[j][:], q1[j][:], cum_kv[j][:])
            oa = [None] * NH
            for j in range(NH):
                bc = ps_pool.tile([128, wg], F32, tag="mm", name=f"bc_{g}_{j}")
                nc.tensor.matmul(
                    bc[:], bcsel_s[:, 128 * j:128 * (j + 1)], den_ir[:, :],
                    start=True, stop=True)
                oa[j] = oa_pool.tile([128, wg], BF16, tag="oa",
                                     name=f"oa_{g}_{j}")
                nc.vector.tensor_mul(oa[j][:], qckv[j][:], bc[:])
            return oa

        def emit_dl(g, oa, vv):
            _, wg = GROUPS[g]
            dls = [None] * NH
            for j in range(NH):
                dls[j] = tmp_pool.tile([128, wg], BF16, tag="dl", bufs=8,
                                       name=f"dl_{g}_{j}")
                nc.gpsimd.tensor_sub(dls[j][:], oa[j][:], vv[j][:])
            return dls

        def emit_gate(g, oa):
            """th = tanh(0.5*L + 0.5*bgate); g = 0.5*th + 0.5."""
            _, wg = GROUPS[g]
            gts = [None] * NH
            for ot in range(NH):
                ps = ps_pool.tile([128, wg], F32, tag="mm", name=f"gp_{g}_{ot}")
                for k in range(NK):
                    nc.tensor.matmul(
                        ps[:], wg_s[k][:, 128 * ot:128 * (ot + 1)], oa[k][:],
                        start=(k == 0), stop=(k == NK - 1))
                gts[ot] = gt_pool.tile([128, wg], BF16, tag="gt",
                                       name=f"gt_{g}_{ot}")
                nc.scalar.activation(
                    gts[ot][:], ps[:], AF.Tanh, scale=0.5,
                    bias=bgate_s[:, ot:ot + 1])
            return gts

        def emit_mix(g, gts, dls, oa):
            # mix = oa + 0.5*(th-1)*(oa-v): d2 = (th-1)*dl on Pool,
            # mix = 0.5*d2 + oa on DVE.
            _, wg = GROUPS[g]
            mix = [None] * NH
            for ot in range(NH):
                d2 = tmp_pool.tile([128, wg], BF16, tag="gd",
                                   name=f"d2_{g}_{ot}")
                nc.vector.scalar_tensor_tensor(
                    d2[:], gts[ot][:], -1.0, dls[ot][:], AL.add, AL.mult)
                mix[ot] = mix_pool.tile([128, wg], BF16, tag="mix",
                                        name=f"mix_{g}_{ot}")
                nc.vector.scalar_tensor_tensor(
                    mix[ot][:], d2[:], 0.5, oa[ot][:], AL.mult, AL.add)
            return mix

        def emit_y(g, mix):
            off, wg = GROUPS[g]
            out_tok = slice(off - HALO, off - HALO + wg)
            for ot in range(NH):
                ps = ps_pool.tile([128, wg], F32, tag="mm", name=f"yp_{g}_{ot}")
                for k in range(NK):
                    nc.tensor.matmul(
                        ps[:], wo_s[k][:, 128 * ot:128 * (ot + 1)], mix[k][:],
                        start=(k == 0), stop=(k == NK - 1))
                ysb = y_pool.tile([128, wg], BF16, tag="ysb", bufs=4,
                                  name=f"ysb_{g}_{ot}")
                nc.scalar.copy(ysb[:], ps[:])
                nc.sync.dma_start(
                    yT.ap()[128 * ot:128 * (ot + 1), out_tok], ysb[:])

        # ---- software-pipelined emission --------------------------------
        # iter g: [xt g+1][qkv-k g][ksum scans g][bc+qckv+oa g-1][qkv-q g]
        #         [prods g][dl g-1][qkv-v g][den g][gate g-1][d2+mix g-1]
        #         [y g-1][kvs+kv scans g]
        # Engine queues this produces (per iter, non-halo):
        #   PE : k(64) bc(8) q(64) v(64) den(8) gate(64) y(64)
        #   DVE: phi-k(8) ksum-scans(8) oa(8) phi-q(8) den-chain d2+mix(16)
        #        kv-scans(8)
        #   Pool: qckv(8) prods(8) dl(8) kvs(8)
        #   ACT: rm/qe-k(16) rm/qe-q(16) v-copies(8) th(8) y-copies(8)
        xts_by_g = {0: xts0, 1: xts_g1}
        prev = None
        for g in range(NG):
            q1 = [None] * NH
            k1 = [None] * NH
            vv = [None] * NH
            if g + 1 < NG and g + 1 not in xts_by_g:
                xts_by_g[g + 1] = emit_xt(g + 1)
            xts = xts_by_g.pop(g)
            emit_qkv(g, xts, 1, q1, k1, vv)      # k-section
            cum_ks = emit_ksum_scans(g, k1)
            if prev is not None:
                p_q1, p_ckv, p_vv, p_den, pg = prev
                oa = emit_oa(pg, p_q1, p_ckv, p_den)
            if g > 0:
                emit_qkv(g, xts, 0, q1, k1, vv)  # q-section
                prods = emit_prods(g, q1, cum_ks)
            if prev is not None:
                dls = emit_dl(pg, oa, p_vv)
            emit_qkv(g, xts, 2, q1, k1, vv)      # v-section
            if prev is not None:
                gts = emit_gate(pg, oa)
                mix = emit_mix(pg, gts, dls, oa)
                emit_y(pg, mix)
            if g > 0:
                dps = emit_den_mm(g, prods)
                den_ir = emit_den_recip(g, dps)
            kvs = emit_kvs(g, k1, vv)
            if prev is not None:
                dep = mix
            elif g > 0:
                dep = prods
            else:
                dep = None
            cum_kv = emit_kv_scans(g, kvs, dep)
            if g > 0:
                prev = (q1, cum_kv, vv, den_ir, g)
        q1, cum_kv, vv, den_ir, g = prev
        oa = emit_oa(g, q1, cum_kv, den_ir)
        dls = emit_dl(g, oa, vv)
        gts = emit_gate(g, oa)
        mix = emit_mix(g, gts, dls, oa)
        emit_y(g, mix)

    nc.compile()
    return nc


def _sigmoid(v):
    return 1.0 / (1.0 + np.exp(-v))


def _make_inputs(x, Wqkv, Wout, Wgate, bgate, decay_param):
    decay = _sigmoid(np.asarray(decay_param, np.float64)).astype(np.float32)
    bf = ml_dtypes.bfloat16
    wqkvT = np.ascontiguousarray(np.asarray(Wqkv, np.float32).T).astype(bf)
    wgateT = np.ascontiguousarray(np.asarray(Wgate, np.float32).T).astype(bf)
    woutT = np.ascontiguousarray(np.asarray(Wout, np.float32).T).astype(bf)

    p = np.arange(128)
    dec_c = np.empty((128, NH), np.float32)
    for j in range(NH):
        dec_c[:, j] = decay[2 * j + p // 64]
    densel = np.zeros((128, NH * H), np.float32)
    for j in range(NH):
        for pp in range(128):
            densel[pp, H * j + 2 * j + pp // 64] = 1.0
    bcsel = np.zeros((H, NH * 128), np.float32)
    for j in range(NH):
        for m in range(128):
            bcsel[2 * j + m // 64, 128 * j + m] = 1.0
    # gate bias rides the tanh drain as ACT bias; ACT computes
    # tanh(0.5*L + bias) so bias must be bgate/2.
    bgate_c = np.ascontiguousarray(
        0.5 * np.asarray(bgate, np.float32).reshape(NH, 128).T)

    in_maps = []
    for c in range(8):
        b, half = c // 2, c % 2
        xb = np.asarray(x[b], np.float32)  # [T, HID]
        if half == 0:
            xloc = np.concatenate(
                [np.zeros((HALO, HID), np.float32), xb[:HALF_T]], axis=0)
            mask = np.zeros((128, 1), np.float32)
        else:
            xloc = xb[HALF_T - HALO:]
            mask = np.ones((128, 1), np.float32)
        in_maps.append({
            "xT": np.ascontiguousarray(xloc.T).astype(bf),
            "wqkvT": wqkvT, "wgateT": wgateT, "woutT": woutT,
            "dec_c": dec_c, "mask_c": mask,
            "densel": densel.astype(bf), "bcsel": bcsel,
            "bgate_c": bgate_c,
        })
    return in_maps


def kernel(x, Wqkv, Wout, Wgate, bgate, decay_param):
    if "nc" not in _cache:
        _cache["nc"] = _build_nc()
    nc = _cache["nc"]
    in_maps = _make_inputs(x, Wqkv, Wout, Wgate, bgate, decay_param)
    res = run_bass_kernel_spmd(nc, in_maps, list(range(8)))
    y = np.empty((B, T, HID), np.float32)
    for c in range(8):
        b, half = c // 2, c % 2
        y[b, half * HALF_T:(half + 1) * HALF_T, :] = (
            res.results[c]["yT"].astype(np.float32).T)
    return y
